# revision 55
# baseline (speedup 1.0000x reference)
"""Multi-head attention (QK-LayerNorm, causal) Trainium2 kernel over 8 NeuronCores.

Sharding: tensor-parallel over heads — 2 heads per core. Each core computes
q/k/v projections for its 128 channels, per-head attention for both batches,
and a partial output projection (its 128-channel slice of Wo); the host sums
the 8 partial bf16 projections in fp64.

Device-side design (HW exec ~156us vs 181us baseline):
- All matmul operands are bf16 (fp32 PSUM accumulation); host pre-converts
  x / weights, and x is laid out [128, D/128, T] so DMA chunks stay >= 512B.
- Projection: ONE full-width [128, 384] matmul per contraction chunk —
  a PSUM bank supports only one accumulation group at a time, so q/k/v
  regions must not carry separate start/stop flags (hardware wipes the
  bank's open group on a new `start`).
- LayerNorm mean-subtraction is folded into the weights on the host; rstd is
  exp(-0.5*ln(var+eps) - 0.5*ln(8)) so q-hat and k-hat each carry 1/sqrt(8)
  and the softmax 1/sqrt(DH) costs nothing. Ln/Exp are batched over the four
  tiles of each x-chunk ([128,16] activations). qk is copied out of PSUM
  immediately (fp32) so the PSUM buffer recycles fast.
- qT comes from the DMA xbar transpose (SBUF->SBUF, bf16); kT from a PE
  transpose + DVE copy — splitting the two halves keeps the SP sequencer's
  serial ~650ns-per-DMA dispatch off the critical path (x-loads are also
  prefetched one chunk ahead for the same reason).
- attn@v is computed transposed: out[q, c] = sum_k ex[k, q] * vaug[k, c]
  (lhsT = ex), streaming 65 columns per (k-tile, q-subtile) instead of the
  q-chunk width; the softmax denominator (ones column of vaug) lands
  per-partition so normalization is a strided reciprocal + one paired
  broadcast multiply per q-subtile. All ex tiles of a chunk stay live and
  each (j, h) runs as one complete PSUM accumulation group (banks alternate
  j order 0,2,1,3) — interleaved groups in one bank are illegal on HW.
- aoT uses a PE transpose (identity matmul) + DVE copy; each chunk's
  out-projection is queued as steps drained inside the next chunk's kt loop
  so its PSUM->SBUF copies never stall the in-order PE stream.
- Causality: fully-masked key tiles are skipped by loop bounds; diagonal
  tiles zero the upper triangle of exp(s) via affine_select (gpsimd).
- Engine placement notes: GPSIMD cannot touch PSUM (walrus verifier), so all
  PSUM reads/writes go to DVE/ACT; Pool keeps SBUF-only work (affine_select,
  qk-hat multiply, memsets). Emission order: b0 projection chunks, then b1
  projection segments interleaved with b0 attention, then b1 attention.
"""

import numpy as np

import concourse.bass as bass
import concourse.mybir as mybir
import concourse.tile as tile
from concourse.bass_utils import run_bass_kernel_spmd
from concourse.masks import make_identity

F32 = mybir.dt.float32
BF16 = mybir.dt.bfloat16

B, S, D, H = 2, 2048, 1024, 16
DH = D // H          # 64
NCORES = 8
HPC = H // NCORES    # 2 heads per core
CH = HPC * DH        # 128 channels per core
T = B * S            # 4096 tokens
DCH = D // 128       # 8 contraction chunks
TT = T // 128        # 32 token tiles
QW = 512             # q-chunk width
QC = S // QW         # 4 q-chunks per batch
KTB = S // 128       # 16 k-tiles per batch
XC = 512             # x-load chunk (tokens)
NXC = T // XC        # 8 x-load chunks
EPS = 1e-5
NEG_HALF_LN8 = -0.5 * float(np.log(8.0))
WS = 32.0            # fp8 weight scale; LN absorbs it for q/k, Wo/WS for v


def _restride(ap, dims):
    """Rebuild an AP keeping its partition dim but replacing the free dims."""
    return bass.AP(tensor=ap.tensor, offset=ap.offset, ap=[ap.ap[0]] + dims)


def _split_drain_waits(nc):
    """walrus in this env only accepts one sync-wait per instruction;
    hoist extra waits onto preceding single-wait NOPs on the same engine."""
    for f in nc.m.functions:
        for blk in f.blocks:
            new_insts = []
            for inst in blk.instructions:
                si = getattr(inst, "sync_info", None)
                if si is not None and si.on_wait and len(si.on_wait) > 1:
                    waits = list(si.on_wait)
                    for j, w in enumerate(waits[:-1]):
                        new_insts.append(
                            mybir.InstNoOp(
                                name=f"{inst.name}-dwsplit{j}",
                                engine=inst.engine,
                                ins=[],
                                outs=[],
                                sync_info=mybir.SyncInfo(on_wait=[w], on_update=[]),
                            )
                        )
                    si.on_wait = [waits[-1]]
                    inst.sync_info = si
                new_insts.append(inst)
            blk.instructions[:] = new_insts


def _build(use_bias=False, dbg=False):
    nc = bass.Bass("TRN2", target_bir_lowering=False, debug=False)
    dbg_d = (
        {
            "qT": nc.dram_tensor("qT_dbg", [128, T], BF16, kind="ExternalOutput"),
            "kT": nc.dram_tensor("kT_dbg", [128, T], BF16, kind="ExternalOutput"),
            "vaug": nc.dram_tensor(
                "vaug_dbg", [128, TT * 2 * (DH + 1)], BF16, kind="ExternalOutput"
            ),
            "rstd": nc.dram_tensor(
                "rstd_dbg", [128, TT * 4], F32, kind="ExternalOutput"
            ),
            "aoT": nc.dram_tensor("aoT_dbg", [128, QW], BF16, kind="ExternalOutput"),
            "qraw": nc.dram_tensor(
                "qraw_dbg", [128, TT * 2 * CH], F32, kind="ExternalOutput"
            ),
        }
        if dbg
        else None
    )

    # x and Wqkv ship as packed (hi, lo) fp8e4m3 pairs: 3 DoubleRow matmuls
    # per 256-row contraction pair compute xh@wh + xh@wl + xl@wh, which is
    # x@w to ~0.1% (better than bf16) at 0.75 PE cycles per 128-chunk row
    # instead of 1.0. Weights carry a x32 scale so they sit in fp8-normal
    # range; LayerNorm absorbs it for q/k and Wo/32 undoes it for v.
    F8 = mybir.dt.float8e4
    xq_d = nc.dram_tensor("xq", [128, DCH * 2 * T], F8, kind="ExternalInput")
    wqkvt_d = nc.dram_tensor(
        "wqkvt", [128, DCH * 2 * 3 * CH], F8, kind="ExternalInput"
    )
    bqkv_d = (
        nc.dram_tensor("bqkv", [1, 3 * CH], F32, kind="ExternalInput")
        if use_bias
        else None
    )
    wot_d = nc.dram_tensor("wot", [CH, D], BF16, kind="ExternalInput")
    pot_d = nc.dram_tensor("pot", [D, T], BF16, kind="ExternalOutput")

    AF = mybir.ActivationFunctionType
    ALU = mybir.AluOpType
    PSW = 3 * CH + 4     # packed projection PSUM tile width

    with tile.TileContext(nc) as tc:
        with (
            tc.tile_pool(name="const", bufs=1) as const_pool,
            tc.tile_pool(name="big", bufs=1) as big,
            tc.tile_pool(name="xt", bufs=4) as xpool,
            tc.tile_pool(name="sq", bufs=8) as sqpool,
            tc.tile_pool(name="ln", bufs=8) as lnpool,
            tc.tile_pool(name="qln", bufs=12) as qlnpool,
            tc.tile_pool(name="expp", bufs=36) as exp_pool,
            tc.tile_pool(name="ao", bufs=12) as ao_pool,
            tc.tile_pool(name="rc", bufs=16) as rc_pool,
            tc.tile_pool(name="aoT", bufs=6) as aoT_pool,
            tc.tile_pool(name="po", bufs=6) as po_pool,
            tc.tile_pool(name="ps_a", bufs=2, space="PSUM") as ps_a_pool,
            tc.tile_pool(name="ps_s", bufs=3, space="PSUM") as ps_s_pool,
            tc.tile_pool(name="ps_po", bufs=1, space="PSUM") as ps_po_pool,
            tc.tile_pool(name="ps_o", bufs=2, space="PSUM") as ps_o_pool,
        ):
            identity = const_pool.tile([128, 128], F32)
            make_identity(nc, identity)
            ident_bf = const_pool.tile([128, 128], BF16)
            nc.gpsimd.tensor_copy(out=ident_bf, in_=identity)
            epsb = const_pool.tile([128, 1], F32)
            nc.gpsimd.memset(epsb, EPS * WS * WS)
            ln8b = const_pool.tile([128, 1], F32)
            nc.gpsimd.memset(ln8b, NEG_HALF_LN8)

            wqkv_sb = const_pool.tile([128, DCH, 2, 3 * CH], F8)

            def emit_wqkv_half(wh):
                hw_ = DCH // 2
                nc.sync.dma_start(
                    out=wqkv_sb[:, wh * hw_ : (wh + 1) * hw_, :, :],
                    in_=wqkvt_d[:, :].rearrange(
                        "p (a c) -> p a c", c=2 * 3 * CH
                    )[:, wh * hw_ : (wh + 1) * hw_, :].rearrange(
                        "p a (l c) -> p a l c", l=2
                    ),
                )

            emit_wqkv_half(0)
            if use_bias:
                bias_sb = const_pool.tile([128, 3 * CH], F32)
                nc.sync.dma_start(
                    out=bias_sb, in_=bqkv_d[0:1, :].to_broadcast([128, 3 * CH])
                )
            qT = big.tile([128, T], BF16)
            kT = big.tile([128, T], BF16)
            vaug = big.tile([128, TT, 2 * (DH + 1)], BF16)
            rstdall = big.tile([128, TT, 4], F32)
            nc.gpsimd.memset(vaug[:, :, DH : DH + 1], 1.0)
            nc.gpsimd.memset(vaug[:, :, 2 * DH + 1 : 2 * DH + 2], 1.0)

            # ---- Phase 1: q/k/v projection + LN ----
            def emit_xload(c):
                xt_sb = xpool.tile(
                    [128, DCH, 2, XC], F8, tag="xt", name="xt_sb"
                )
                xv = xq_d[:, :].rearrange("p (a l t) -> p a l t", l=2, t=T)
                if c == 0:
                    # hi-plane first: the (xh,wh)/(xh,wl) matmul terms only
                    # need the hi plane, so phase-1 starts after half the load
                    for plane in range(2):
                        nc.sync.dma_start(
                            out=xt_sb[:, :, plane, :],
                            in_=xv[:, :, plane, XC * c : XC * (c + 1)],
                        )
                    return xt_sb
                hd = DCH // 2
                for part in range(2):
                    nc.sync.dma_start(
                        out=xt_sb[:, part * hd : (part + 1) * hd, :, :],
                        in_=xv[
                            :, part * hd : (part + 1) * hd, :,
                            XC * c : XC * (c + 1),
                        ],
                    )
                return xt_sb

            def emit_proj_tile(xt_sb, t, ssq4):
                    tt = t % (XC // 128)
                    DR = mybir.MatmulPerfMode.DoubleRow
                    ps = ps_a_pool.tile([128, 3 * CH], F32, tag="a", name="ps")
                    NP = DCH // 2
                    # phase 1 consumes only the hi plane of x; the xl terms
                    # come last so the lo-plane DMA can lag
                    terms = []
                    for p in range(NP):
                        xh = xt_sb[:, 2 * p : 2 * p + 2, 0,
                                   128 * tt : 128 * (tt + 1)]
                        wh = wqkv_sb[:, 2 * p : 2 * p + 2, 0, :]
                        wl = wqkv_sb[:, 2 * p : 2 * p + 2, 1, :]
                        terms += [(xh, wh), (xh, wl)]
                    for p in range(NP):
                        xl = xt_sb[:, 2 * p : 2 * p + 2, 1,
                                   128 * tt : 128 * (tt + 1)]
                        wh = wqkv_sb[:, 2 * p : 2 * p + 2, 0, :]
                        terms.append((xl, wh))
                    for ti, (lt, rt) in enumerate(terms):
                        nc.tensor.matmul(
                            ps,
                            lhsT=lt,
                            rhs=rt,
                            start=(ti == 0),
                            stop=(ti == len(terms) - 1),
                            perf_mode=DR,
                        )
                    if use_bias:
                        nc.vector.tensor_add(out=ps, in0=ps, in1=bias_sb)

                    # qk is copied out of PSUM (bf16) right away: the ps
                    # buffer frees early and the whole LN chain (square,
                    # grouped reduce, qk-hat multiply) runs SBUF-only on the
                    # Pool engine, which cannot touch PSUM.
                    qkcp = qlnpool.tile([128, 2 * CH], F32, tag="qkcp", name="qkcp")
                    nc.vector.tensor_copy(out=qkcp, in_=ps[:, 0 : 2 * CH])
                    if dbg:
                        nc.sync.dma_start(
                            out=dbg_d["qraw"][:, 2 * CH * t : 2 * CH * (t + 1)],
                            in_=qkcp,
                        )
                    sq = sqpool.tile([128, 2 * CH], F32, tag="sq", name="sq")
                    nc.scalar.activation(out=sq, in_=qkcp, func=AF.Square)
                    nc.vector.reduce_sum(
                        out=ssq4[:, 4 * (t % 4) : 4 * (t % 4) + 4],
                        in_=sq.rearrange("p (g x) -> p g x", x=DH),
                        axis=mybir.AxisListType.X,
                    )
                    # v into vaug cols [0:64] and [65:129] (one strided copy)
                    vdst = vaug[:, t, 0:1]
                    nc.vector.tensor_copy(
                        out=_restride(vdst, [[DH + 1, HPC], [1, DH]]),
                        in_=ps[:, 2 * CH : 3 * CH].rearrange(
                            "p (h x) -> p h x", x=DH
                        ),
                    )
                    return qkcp

            def emit_proj_back(qkcp, t):
                    # (q,k)-hat = (q,k) * rstd/sqrt(8), bf16 for the PE transposes
                    qkln = qlnpool.tile([128, 2 * CH], BF16, tag="qkln", name="qkln")
                    rq = rstdall[:, t, 0:4]
                    nc.gpsimd.tensor_mul(
                        out=qkln.rearrange("p (g x) -> p g x", x=DH),
                        in0=qkcp.rearrange("p (g x) -> p g x", x=DH),
                        in1=bass.AP(
                            tensor=rq.tensor, offset=rq.offset, ap=rq.ap + [[0, DH]]
                        ),
                    )
                    nc.sync.dma_start_transpose(
                        out=qT[:, 128 * t : 128 * (t + 1)], in_=qkln[:, 0:CH]
                    )
                    tk = ps_po_pool.tile([128, 128], BF16, tag="p", name="tk")
                    nc.tensor.transpose(tk, qkln[:, CH : 2 * CH], ident_bf)
                    nc.vector.tensor_copy(
                        out=kT[:, 128 * t : 128 * (t + 1)], in_=tk
                    )

            # ---- Phase 2: per-head causal attention + partial out-projection ----
            def emit_attn_chunk(b, qc, op_steps):
                    q0 = b * S + qc * QW
                    n_kt = (qc + 1) * (QW // 128)
                    pso0 = ps_o_pool.tile([128, 2 * 2 * (DH + 1)], F32, tag="o")
                    pso1 = ps_o_pool.tile([128, 2 * 2 * (DH + 1)], F32, tag="o")
                    pso = [pso0, pso1]
                    exs = {}
                    kt_i = 0
                    for kt in range(n_kt):
                        kt_i += 1
                        dg = kt - qc * (QW // 128)
                        c0 = max(0, 128 * dg)
                        ktg = b * KTB + kt
                        for h in range(HPC):
                            ps_s = ps_s_pool.tile(
                                [128, QW], F32, tag="s", name="ps_s"
                            )
                            nc.tensor.matmul(
                                ps_s[:, c0:QW],
                                lhsT=kT[
                                    DH * h : DH * (h + 1),
                                    128 * ktg : 128 * (ktg + 1),
                                ],
                                rhs=qT[DH * h : DH * (h + 1), q0 + c0 : q0 + QW],
                                start=True,
                                stop=True,
                            )
                            ex = exp_pool.tile(
                                [128, QW], BF16, tag="ex", name="ex"
                            )
                            nc.scalar.activation(
                                out=ex[:, c0:QW], in_=ps_s[:, c0:QW], func=AF.Exp
                            )
                            if dg >= 0:
                                nc.gpsimd.affine_select(
                                    out=ex[:, c0 : c0 + 128],
                                    in_=ex[:, c0 : c0 + 128],
                                    compare_op=ALU.is_ge,
                                    fill=0.0,
                                    base=0,
                                    pattern=[[1, 128]],
                                    channel_multiplier=-1,
                                )
                            exs[(kt, h)] = ex
                        if kt_i > 2 and op_steps:
                            op_steps.pop(0)()
                            if op_steps:
                                op_steps.pop(0)()
                    # one complete accumulation group per (j, h); banks
                    # alternate (j order 0,2,1,3) so one bank's normalize
                    # overlaps the other bank's accumulation
                    aoT = aoT_pool.tile([128, QW], BF16, tag="aoT", name="aoT")
                    for j in (0, 2, 1, 3):
                        jj, jo = divmod(j, 2)
                        last_kt = qc * (QW // 128) + j
                        for h in range(HPC):
                            off = 2 * (DH + 1) * jo + (DH + 1) * h
                            for kt in range(last_kt, -1, -1):
                                nc.tensor.matmul(
                                    pso[jj][:, off : off + DH + 1],
                                    lhsT=exs[(kt, h)][
                                        :, 128 * j : 128 * (j + 1)
                                    ],
                                    rhs=vaug[
                                        :,
                                        b * KTB + kt,
                                        (DH + 1) * h : (DH + 1) * (h + 1),
                                    ],
                                    start=(kt == last_kt),
                                    stop=(kt == 0),
                                )
                            if op_steps:
                                op_steps.pop(0)()
                        # normalize this q-subtile: strided reciprocal over
                        # both heads, one paired broadcast multiply, PE
                        # transpose into the aoT staging tile
                        base = 2 * (DH + 1) * jo
                        den = pso[jj][:, base + DH : base + DH + 1]
                        rcp = rc_pool.tile([128, HPC], F32, tag="rcp", name="rcp")
                        nc.vector.reciprocal(
                            out=rcp, in_=_restride(den, [[DH + 1, HPC]])
                        )
                        ao = ao_pool.tile([128, CH], BF16, tag="ao", name="ao")
                        num = pso[jj][:, base : base + 1]
                        rp = rcp[:, 0:1]
                        nc.vector.tensor_mul(
                            out=ao.rearrange("p (h x) -> p h x", x=DH),
                            in0=_restride(num, [[DH + 1, HPC], [1, DH]]),
                            in1=_restride(rp, [[1, HPC], [0, DH]]),
                        )
                        tao = ps_po_pool.tile([128, 128], BF16, tag="p", name="tao")
                        nc.tensor.transpose(tao, ao, ident_bf)
                        nc.vector.tensor_copy(
                            out=aoT[:, 128 * j : 128 * (j + 1)], in_=tao
                        )
                    return aoT, q0

            def outproj_steps(aoT, q0, mm_pool=None, split=False):
                po_sb = po_pool.tile([128, DCH, QW], BF16, tag="po", name="po_sb")
                hd = DCH // 2

                def mk_mm(dc):
                    def step():
                        pool = mm_pool or ps_po_pool
                        ps_po = pool.tile(
                            [128, QW],
                            F32,
                            tag="o" if mm_pool is not None else "p",
                            name="ps_po",
                        )
                        nc.tensor.matmul(
                            ps_po,
                            lhsT=wo_sb[:, 128 * dc : 128 * (dc + 1)],
                            rhs=aoT,
                            start=True,
                            stop=True,
                        )
                        # steady-state drains stay on DVE (ACT is exp-bound);
                        # the tail (after the last attention chunk) splits
                        # DVE/ACT since ACT idles there
                        if split and dc % 2 == 1:
                            nc.scalar.copy(out=po_sb[:, dc, :], in_=ps_po)
                        else:
                            nc.vector.tensor_copy(out=po_sb[:, dc, :], in_=ps_po)

                    return step

                def mk_dma(half):
                    def step():
                        nc.sync.dma_start(
                            out=pot_d[:, :].rearrange("(a p) t -> p a t", p=128)[
                                :, half * hd : (half + 1) * hd, q0 : q0 + QW
                            ],
                            in_=po_sb[:, half * hd : (half + 1) * hd, :],
                        )

                    return step

                steps = []
                for half in range(2):
                    for dc in range(half * hd, (half + 1) * hd):
                        steps.append(mk_mm(dc))
                    steps.append(mk_dma(half))
                return steps

            # ---- Emission order: b0 proj, then b1 proj segments
            # interleaved with b0 attention chunks, then b1 attention.
            # Each chunk's out-projection is deferred past the next chunk's
            # kt loop so its PSUM->SBUF copies never block the PE stream.
            NTC = XC // 128

            xts = {}

            def get_xload(c):
                if c < NXC and c not in xts:
                    xts[c] = emit_xload(c)
                return xts.get(c)

            def emit_proj_chunk(c):
                xt_sb = get_xload(c)
                if c == 0:
                    emit_wqkv_half(1)
                get_xload(c + 1)
                ssq4 = lnpool.tile([128, 4 * NTC], F32, tag="ssq", name="ssq4")
                qkcps = [
                    emit_proj_tile(xt_sb, NTC * c + tt, ssq4) for tt in range(NTC)
                ]
                lnv = lnpool.tile([128, 4 * NTC], F32, tag="lnv", name="lnv")
                nc.scalar.activation(
                    out=lnv, in_=ssq4, func=AF.Ln, scale=1.0 / DH, bias=epsb[:, :]
                )
                nc.scalar.activation(
                    out=rstdall[:, NTC * c : NTC * (c + 1), :].rearrange(
                        "p a b -> p (a b)"
                    ),
                    in_=lnv,
                    func=AF.Exp,
                    scale=-0.5,
                    bias=ln8b[:, :],
                )
                for tt in range(NTC):
                    emit_proj_back(qkcps[tt], NTC * c + tt)

            # pipelined emission with a LAG: attention chunk i is emitted
            # after projection chunk i+LAG, so the exp stream starts early
            # but each chunk's LN-chain -> kT latency is hidden by the
            # in-flight projection work ahead of it
            LAG = 2
            wo_sb = const_pool.tile([CH, D], BF16, name="wo_sb")
            steps = []
            last_aoT = [None]
            NCH = B * QC

            def attn_of(i):
                return (i // QC, i % QC)

            for i in range(NCH + LAG):
                if i < NCH:
                    emit_proj_chunk(i)
                    if i == 0:
                        nc.sync.dma_start(out=wo_sb, in_=wot_d[:, :])
                ai = i - LAG
                if ai < 0:
                    continue
                b, qc = attn_of(ai)
                st = emit_attn_chunk(b, qc, steps)
                last_aoT[0] = st
                for s in steps:
                    s()
                if ai == NCH - 1:
                    steps = outproj_steps(*st, mm_pool=ps_o_pool, split=True)
                else:
                    steps = outproj_steps(*st)
            for s in steps:
                s()
            if dbg:
                nc.sync.dma_start(out=dbg_d["qT"][:, :], in_=qT)
                nc.sync.dma_start(out=dbg_d["kT"][:, :], in_=kT)
                nc.sync.dma_start(
                    out=dbg_d["vaug"][:, :],
                    in_=vaug.rearrange("p a b -> p (a b)"),
                )
                nc.sync.dma_start(
                    out=dbg_d["rstd"][:, :],
                    in_=rstdall.rearrange("p a b -> p (a b)"),
                )
                nc.sync.dma_start(out=dbg_d["aoT"][:, :], in_=last_aoT[0][0])

    _split_drain_waits(nc)
    return nc


_NC_CACHE = {}


def _get_nc(use_bias=False, dbg=False):
    key = (use_bias, dbg)
    if key not in _NC_CACHE:
        _NC_CACHE[key] = _build(use_bias, dbg=dbg)
    return _NC_CACHE[key]


def _bf16(a):
    return np.asarray(a, dtype=np.float32).astype(mybir.dt.np(BF16))


def _f8_pair(a):
    """Split fp32 array into (hi, lo) fp8e4m3 with hi+lo ~= a (0.05% resid)."""
    f8 = mybir.dt.np(mybir.dt.float8e4)
    hi = a.astype(f8)
    lo = (a - hi.astype(np.float32)).astype(f8)
    return hi, lo


def _prep_inputs(x, Wq, bq, Wk, bk, Wv, bv, Wo):
    xt = np.ascontiguousarray(x.reshape(T, D).T).astype(np.float32)
    xc = xt.reshape(DCH, 128, T).transpose(1, 0, 2)  # [128, DCH, T]
    xh, xl = _f8_pair(xc)
    xq = np.ascontiguousarray(
        np.stack([xh, xl], axis=2).reshape(128, DCH * 2 * T)
    )
    in_maps = []
    for c in range(NCORES):
        sl = slice(CH * c, CH * (c + 1))
        wq_c = np.array(Wq[sl, :], dtype=np.float32)
        bq_c = np.array(bq[sl], dtype=np.float32)
        wk_c = np.array(Wk[sl, :], dtype=np.float32)
        bk_c = np.array(bk[sl], dtype=np.float32)
        # fold the LayerNorm mean-subtraction (a linear map) into W and b
        for h in range(HPC):
            blk = slice(DH * h, DH * (h + 1))
            wq_c[blk, :] -= wq_c[blk, :].mean(axis=0, keepdims=True)
            bq_c[blk] -= bq_c[blk].mean()
            wk_c[blk, :] -= wk_c[blk, :].mean(axis=0, keepdims=True)
            bk_c[blk] -= bk_c[blk].mean()
        wv_c = np.array(Wv[sl, :], dtype=np.float32)
        bv_c = np.array(bv[sl], dtype=np.float32)
        wfull = np.ascontiguousarray(
            np.concatenate([wq_c, wk_c, wv_c], axis=0).T * WS
        )  # [D, 3CH], x32 into fp8-normal range
        wc = wfull.reshape(DCH, 128, 3 * CH).transpose(1, 0, 2)
        wh, wl = _f8_pair(wc)
        wqkvt = np.ascontiguousarray(
            np.stack([wh, wl], axis=2).reshape(128, DCH * 2 * 3 * CH)
        )
        bqkv = (np.concatenate([bq_c, bk_c, bv_c]) * WS)[None, :].astype(
            np.float32
        )
        wot = _bf16(np.ascontiguousarray(Wo[:, sl].T) / WS)
        in_maps.append({"xq": xq, "wqkvt": wqkvt, "bqkv": bqkv, "wot": wot})
    return in_maps


def kernel(x, mask, Wq, bq, Wk, bk, Wv, bv, Wo, bo, _trace=False):
    x = np.asarray(x, dtype=np.float32)
    in_maps = _prep_inputs(
        x,
        np.asarray(Wq),
        np.asarray(bq),
        np.asarray(Wk),
        np.asarray(bk),
        np.asarray(Wv),
        np.asarray(bv),
        np.asarray(Wo),
    )
    use_bias = bool(
        np.any(np.asarray(bq)) or np.any(np.asarray(bk)) or np.any(np.asarray(bv))
    )
    if not use_bias:
        for m in in_maps:
            del m["bqkv"]
    nc = _get_nc(use_bias)
    res = run_bass_kernel_spmd(
        nc, in_maps, core_ids=list(range(NCORES)), trace=_trace
    )
    pot = np.zeros((D, T), np.float64)
    for c in range(NCORES):
        pot += res.results[c]["pot"].astype(np.float64)
    out = pot.T.astype(np.float32) + np.asarray(bo, dtype=np.float32)[None, :]
    out = out.reshape(B, S, D)
    if _trace:
        return out, res
    return out



# revision 57
# speedup vs baseline: 1.0035x; 1.0035x over previous
"""Multi-head attention (QK-LayerNorm, causal) Trainium2 kernel over 8 NeuronCores.

Sharding: tensor-parallel over heads — 2 heads per core. Each core computes
q/k/v projections for its 128 channels, per-head attention for both batches,
and a partial output projection (its 128-channel slice of Wo); the host sums
the 8 partial bf16 projections in fp64.

Device-side design (HW exec ~156us vs 181us baseline):
- All matmul operands are bf16 (fp32 PSUM accumulation); host pre-converts
  x / weights, and x is laid out [128, D/128, T] so DMA chunks stay >= 512B.
- Projection: ONE full-width [128, 384] matmul per contraction chunk —
  a PSUM bank supports only one accumulation group at a time, so q/k/v
  regions must not carry separate start/stop flags (hardware wipes the
  bank's open group on a new `start`).
- LayerNorm mean-subtraction is folded into the weights on the host; rstd is
  exp(-0.5*ln(var+eps) - 0.5*ln(8)) so q-hat and k-hat each carry 1/sqrt(8)
  and the softmax 1/sqrt(DH) costs nothing. Ln/Exp are batched over the four
  tiles of each x-chunk ([128,16] activations). qk is copied out of PSUM
  immediately (fp32) so the PSUM buffer recycles fast.
- qT comes from the DMA xbar transpose (SBUF->SBUF, bf16); kT from a PE
  transpose + DVE copy — splitting the two halves keeps the SP sequencer's
  serial ~650ns-per-DMA dispatch off the critical path (x-loads are also
  prefetched one chunk ahead for the same reason).
- attn@v is computed transposed: out[q, c] = sum_k ex[k, q] * vaug[k, c]
  (lhsT = ex), streaming 65 columns per (k-tile, q-subtile) instead of the
  q-chunk width; the softmax denominator (ones column of vaug) lands
  per-partition so normalization is a strided reciprocal + one paired
  broadcast multiply per q-subtile. All ex tiles of a chunk stay live and
  each (j, h) runs as one complete PSUM accumulation group (banks alternate
  j order 0,2,1,3) — interleaved groups in one bank are illegal on HW.
- aoT uses a PE transpose (identity matmul) + DVE copy; each chunk's
  out-projection is queued as steps drained inside the next chunk's kt loop
  so its PSUM->SBUF copies never stall the in-order PE stream.
- Causality: fully-masked key tiles are skipped by loop bounds; diagonal
  tiles zero the upper triangle of exp(s) via affine_select (gpsimd).
- Engine placement notes: GPSIMD cannot touch PSUM (walrus verifier), so all
  PSUM reads/writes go to DVE/ACT; Pool keeps SBUF-only work (affine_select,
  qk-hat multiply, memsets). Emission order: b0 projection chunks, then b1
  projection segments interleaved with b0 attention, then b1 attention.
"""

import numpy as np

import concourse.bass as bass
import concourse.mybir as mybir
import concourse.tile as tile
from concourse.bass_utils import run_bass_kernel_spmd
from concourse.masks import make_identity

F32 = mybir.dt.float32
BF16 = mybir.dt.bfloat16

B, S, D, H = 2, 2048, 1024, 16
DH = D // H          # 64
NCORES = 8
HPC = H // NCORES    # 2 heads per core
CH = HPC * DH        # 128 channels per core
T = B * S            # 4096 tokens
DCH = D // 128       # 8 contraction chunks
TT = T // 128        # 32 token tiles
QW = 512             # q-chunk width
QC = S // QW         # 4 q-chunks per batch
KTB = S // 128       # 16 k-tiles per batch
XC = 512             # x-load chunk (tokens)
NXC = T // XC        # 8 x-load chunks
EPS = 1e-5
NEG_HALF_LN8 = -0.5 * float(np.log(8.0))
WS = 32.0            # fp8 weight scale; LN absorbs it for q/k, Wo/WS for v


def _restride(ap, dims):
    """Rebuild an AP keeping its partition dim but replacing the free dims."""
    return bass.AP(tensor=ap.tensor, offset=ap.offset, ap=[ap.ap[0]] + dims)


def _split_drain_waits(nc):
    """walrus in this env only accepts one sync-wait per instruction;
    hoist extra waits onto preceding single-wait NOPs on the same engine."""
    for f in nc.m.functions:
        for blk in f.blocks:
            new_insts = []
            for inst in blk.instructions:
                si = getattr(inst, "sync_info", None)
                if si is not None and si.on_wait and len(si.on_wait) > 1:
                    waits = list(si.on_wait)
                    for j, w in enumerate(waits[:-1]):
                        new_insts.append(
                            mybir.InstNoOp(
                                name=f"{inst.name}-dwsplit{j}",
                                engine=inst.engine,
                                ins=[],
                                outs=[],
                                sync_info=mybir.SyncInfo(on_wait=[w], on_update=[]),
                            )
                        )
                    si.on_wait = [waits[-1]]
                    inst.sync_info = si
                new_insts.append(inst)
            blk.instructions[:] = new_insts


def _build(use_bias=False, dbg=False):
    nc = bass.Bass("TRN2", target_bir_lowering=False, debug=False)
    dbg_d = (
        {
            "qT": nc.dram_tensor("qT_dbg", [128, T], BF16, kind="ExternalOutput"),
            "kT": nc.dram_tensor("kT_dbg", [128, T], BF16, kind="ExternalOutput"),
            "vaug": nc.dram_tensor(
                "vaug_dbg", [128, TT * 2 * (DH + 1)], BF16, kind="ExternalOutput"
            ),
            "rstd": nc.dram_tensor(
                "rstd_dbg", [128, TT * 4], F32, kind="ExternalOutput"
            ),
            "aoT": nc.dram_tensor("aoT_dbg", [128, QW], BF16, kind="ExternalOutput"),
            "qraw": nc.dram_tensor(
                "qraw_dbg", [128, TT * 2 * CH], F32, kind="ExternalOutput"
            ),
        }
        if dbg
        else None
    )

    # x and Wqkv ship as packed (hi, lo) fp8e4m3 pairs: 3 DoubleRow matmuls
    # per 256-row contraction pair compute xh@wh + xh@wl + xl@wh, which is
    # x@w to ~0.1% (better than bf16) at 0.75 PE cycles per 128-chunk row
    # instead of 1.0. Weights carry a x32 scale so they sit in fp8-normal
    # range; LayerNorm absorbs it for q/k and Wo/32 undoes it for v.
    F8 = mybir.dt.float8e4
    xq_d = nc.dram_tensor("xq", [128, DCH * 2 * T], F8, kind="ExternalInput")
    wqkvt_d = nc.dram_tensor(
        "wqkvt", [128, DCH * 2 * 3 * CH], F8, kind="ExternalInput"
    )
    bqkv_d = (
        nc.dram_tensor("bqkv", [1, 3 * CH], F32, kind="ExternalInput")
        if use_bias
        else None
    )
    wot_d = nc.dram_tensor("wot", [CH, D], BF16, kind="ExternalInput")
    pot_d = nc.dram_tensor("pot", [D, T], BF16, kind="ExternalOutput")

    AF = mybir.ActivationFunctionType
    ALU = mybir.AluOpType
    PSW = 3 * CH + 4     # packed projection PSUM tile width

    with tile.TileContext(nc) as tc:
        with (
            tc.tile_pool(name="const", bufs=1) as const_pool,
            tc.tile_pool(name="big", bufs=1) as big,
            tc.tile_pool(name="xt", bufs=4) as xpool,
            tc.tile_pool(name="sq", bufs=8) as sqpool,
            tc.tile_pool(name="ln", bufs=8) as lnpool,
            tc.tile_pool(name="qln", bufs=12) as qlnpool,
            tc.tile_pool(name="expp", bufs=36) as exp_pool,
            tc.tile_pool(name="ao", bufs=12) as ao_pool,
            tc.tile_pool(name="rc", bufs=16) as rc_pool,
            tc.tile_pool(name="aoT", bufs=6) as aoT_pool,
            tc.tile_pool(name="po", bufs=6) as po_pool,
            tc.tile_pool(name="ps_a", bufs=2, space="PSUM") as ps_a_pool,
            tc.tile_pool(name="ps_s", bufs=3, space="PSUM") as ps_s_pool,
            tc.tile_pool(name="ps_po", bufs=1, space="PSUM") as ps_po_pool,
            tc.tile_pool(name="ps_o", bufs=2, space="PSUM") as ps_o_pool,
        ):
            identity = const_pool.tile([128, 128], F32)
            make_identity(nc, identity)
            ident_bf = const_pool.tile([128, 128], BF16)
            nc.gpsimd.tensor_copy(out=ident_bf, in_=identity)
            epsb = const_pool.tile([128, 1], F32)
            nc.gpsimd.memset(epsb, EPS * WS * WS)
            ln8b = const_pool.tile([128, 1], F32)
            nc.gpsimd.memset(ln8b, NEG_HALF_LN8)

            wqkv_sb = const_pool.tile([128, DCH, 2, 3 * CH], F8)

            def emit_wqkv_half(wh):
                hw_ = DCH // 2
                nc.sync.dma_start(
                    out=wqkv_sb[:, wh * hw_ : (wh + 1) * hw_, :, :],
                    in_=wqkvt_d[:, :].rearrange(
                        "p (a c) -> p a c", c=2 * 3 * CH
                    )[:, wh * hw_ : (wh + 1) * hw_, :].rearrange(
                        "p a (l c) -> p a l c", l=2
                    ),
                )

            emit_wqkv_half(0)
            if use_bias:
                bias_sb = const_pool.tile([128, 3 * CH], F32)
                nc.sync.dma_start(
                    out=bias_sb, in_=bqkv_d[0:1, :].to_broadcast([128, 3 * CH])
                )
            qT = big.tile([128, T], BF16)
            kT = big.tile([128, T], BF16)
            vaug = big.tile([128, TT, 2 * (DH + 1)], BF16)
            rstdall = big.tile([128, TT, 4], F32)
            nc.gpsimd.memset(vaug[:, :, DH : DH + 1], 1.0)
            nc.gpsimd.memset(vaug[:, :, 2 * DH + 1 : 2 * DH + 2], 1.0)

            # ---- Phase 1: q/k/v projection + LN ----
            def emit_xload(c):
                xt_sb = xpool.tile(
                    [128, DCH, 2, XC], F8, tag="xt", name="xt_sb"
                )
                xv = xq_d[:, :].rearrange("p (a l t) -> p a l t", l=2, t=T)
                if c == 0:
                    # hi-plane first: the (xh,wh)/(xh,wl) matmul terms only
                    # need the hi plane, so phase-1 starts after half the load
                    for plane in range(2):
                        nc.sync.dma_start(
                            out=xt_sb[:, :, plane, :],
                            in_=xv[:, :, plane, XC * c : XC * (c + 1)],
                        )
                    return xt_sb
                nc.sync.dma_start(
                    out=xt_sb[:, :, :, :],
                    in_=xv[:, :, :, XC * c : XC * (c + 1)],
                )
                return xt_sb

            def emit_proj_tile(xt_sb, t, ssq4):
                    tt = t % (XC // 128)
                    DR = mybir.MatmulPerfMode.DoubleRow
                    ps = ps_a_pool.tile([128, 3 * CH], F32, tag="a", name="ps")
                    NP = DCH // 2
                    # phase 1 consumes only the hi plane of x; the xl terms
                    # come last so the lo-plane DMA can lag
                    terms = []
                    for p in range(NP):
                        xh = xt_sb[:, 2 * p : 2 * p + 2, 0,
                                   128 * tt : 128 * (tt + 1)]
                        wh = wqkv_sb[:, 2 * p : 2 * p + 2, 0, :]
                        wl = wqkv_sb[:, 2 * p : 2 * p + 2, 1, :]
                        terms += [(xh, wh), (xh, wl)]
                    for p in range(NP):
                        xl = xt_sb[:, 2 * p : 2 * p + 2, 1,
                                   128 * tt : 128 * (tt + 1)]
                        wh = wqkv_sb[:, 2 * p : 2 * p + 2, 0, :]
                        terms.append((xl, wh))
                    for ti, (lt, rt) in enumerate(terms):
                        nc.tensor.matmul(
                            ps,
                            lhsT=lt,
                            rhs=rt,
                            start=(ti == 0),
                            stop=(ti == len(terms) - 1),
                            perf_mode=DR,
                        )
                    if use_bias:
                        nc.vector.tensor_add(out=ps, in0=ps, in1=bias_sb)

                    # qk is copied out of PSUM (bf16) right away: the ps
                    # buffer frees early and the whole LN chain (square,
                    # grouped reduce, qk-hat multiply) runs SBUF-only on the
                    # Pool engine, which cannot touch PSUM.
                    qkcp = qlnpool.tile([128, 2 * CH], F32, tag="qkcp", name="qkcp")
                    nc.vector.tensor_copy(out=qkcp, in_=ps[:, 0 : 2 * CH])
                    if dbg:
                        nc.sync.dma_start(
                            out=dbg_d["qraw"][:, 2 * CH * t : 2 * CH * (t + 1)],
                            in_=qkcp,
                        )
                    sq = sqpool.tile([128, 2 * CH], F32, tag="sq", name="sq")
                    nc.scalar.activation(out=sq, in_=qkcp, func=AF.Square)
                    nc.vector.reduce_sum(
                        out=ssq4[:, 4 * (t % 4) : 4 * (t % 4) + 4],
                        in_=sq.rearrange("p (g x) -> p g x", x=DH),
                        axis=mybir.AxisListType.X,
                    )
                    # v into vaug cols [0:64] and [65:129] (one strided copy)
                    vdst = vaug[:, t, 0:1]
                    nc.vector.tensor_copy(
                        out=_restride(vdst, [[DH + 1, HPC], [1, DH]]),
                        in_=ps[:, 2 * CH : 3 * CH].rearrange(
                            "p (h x) -> p h x", x=DH
                        ),
                    )
                    return qkcp

            def emit_proj_back(qkcp, t):
                    # (q,k)-hat = (q,k) * rstd/sqrt(8), bf16 for the PE transposes
                    qkln = qlnpool.tile([128, 2 * CH], BF16, tag="qkln", name="qkln")
                    rq = rstdall[:, t, 0:4]
                    nc.gpsimd.tensor_mul(
                        out=qkln.rearrange("p (g x) -> p g x", x=DH),
                        in0=qkcp.rearrange("p (g x) -> p g x", x=DH),
                        in1=bass.AP(
                            tensor=rq.tensor, offset=rq.offset, ap=rq.ap + [[0, DH]]
                        ),
                    )
                    nc.sync.dma_start_transpose(
                        out=qT[:, 128 * t : 128 * (t + 1)], in_=qkln[:, 0:CH]
                    )
                    tk = ps_po_pool.tile([128, 128], BF16, tag="p", name="tk")
                    nc.tensor.transpose(tk, qkln[:, CH : 2 * CH], ident_bf)
                    nc.vector.tensor_copy(
                        out=kT[:, 128 * t : 128 * (t + 1)], in_=tk
                    )

            # ---- Phase 2: per-head causal attention + partial out-projection ----
            def emit_attn_chunk(b, qc, op_steps):
                    q0 = b * S + qc * QW
                    n_kt = (qc + 1) * (QW // 128)
                    pso0 = ps_o_pool.tile([128, 2 * 2 * (DH + 1)], F32, tag="o")
                    pso1 = ps_o_pool.tile([128, 2 * 2 * (DH + 1)], F32, tag="o")
                    pso = [pso0, pso1]
                    exs = {}
                    kt_i = 0
                    for kt in range(n_kt - 1, -1, -1):
                        kt_i += 1
                        dg = kt - qc * (QW // 128)
                        c0 = max(0, 128 * dg)
                        ktg = b * KTB + kt
                        for h in range(HPC):
                            ps_s = ps_s_pool.tile(
                                [128, QW], F32, tag="s", name="ps_s"
                            )
                            nc.tensor.matmul(
                                ps_s[:, c0:QW],
                                lhsT=kT[
                                    DH * h : DH * (h + 1),
                                    128 * ktg : 128 * (ktg + 1),
                                ],
                                rhs=qT[DH * h : DH * (h + 1), q0 + c0 : q0 + QW],
                                start=True,
                                stop=True,
                            )
                            ex = exp_pool.tile(
                                [128, QW], BF16, tag="ex", name="ex"
                            )
                            nc.scalar.activation(
                                out=ex[:, c0:QW], in_=ps_s[:, c0:QW], func=AF.Exp
                            )
                            if dg >= 0:
                                nc.gpsimd.affine_select(
                                    out=ex[:, c0 : c0 + 128],
                                    in_=ex[:, c0 : c0 + 128],
                                    compare_op=ALU.is_ge,
                                    fill=0.0,
                                    base=0,
                                    pattern=[[1, 128]],
                                    channel_multiplier=-1,
                                )
                            exs[(kt, h)] = ex
                        if kt_i > 2 and op_steps:
                            op_steps.pop(0)()
                            if op_steps:
                                op_steps.pop(0)()
                    # one complete accumulation group per (j, h); banks
                    # alternate (j order 0,2,1,3) so one bank's normalize
                    # overlaps the other bank's accumulation
                    aoT = aoT_pool.tile([128, QW], BF16, tag="aoT", name="aoT")
                    for j in (0, 2, 1, 3):
                        jj, jo = divmod(j, 2)
                        last_kt = qc * (QW // 128) + j
                        for h in range(HPC):
                            off = 2 * (DH + 1) * jo + (DH + 1) * h
                            for kt in range(last_kt, -1, -1):
                                nc.tensor.matmul(
                                    pso[jj][:, off : off + DH + 1],
                                    lhsT=exs[(kt, h)][
                                        :, 128 * j : 128 * (j + 1)
                                    ],
                                    rhs=vaug[
                                        :,
                                        b * KTB + kt,
                                        (DH + 1) * h : (DH + 1) * (h + 1),
                                    ],
                                    start=(kt == last_kt),
                                    stop=(kt == 0),
                                )
                            if op_steps:
                                op_steps.pop(0)()
                        # normalize this q-subtile: strided reciprocal over
                        # both heads, one paired broadcast multiply, PE
                        # transpose into the aoT staging tile
                        base = 2 * (DH + 1) * jo
                        den = pso[jj][:, base + DH : base + DH + 1]
                        rcp = rc_pool.tile([128, HPC], F32, tag="rcp", name="rcp")
                        nc.vector.reciprocal(
                            out=rcp, in_=_restride(den, [[DH + 1, HPC]])
                        )
                        ao = ao_pool.tile([128, CH], BF16, tag="ao", name="ao")
                        num = pso[jj][:, base : base + 1]
                        rp = rcp[:, 0:1]
                        nc.vector.tensor_mul(
                            out=ao.rearrange("p (h x) -> p h x", x=DH),
                            in0=_restride(num, [[DH + 1, HPC], [1, DH]]),
                            in1=_restride(rp, [[1, HPC], [0, DH]]),
                        )
                        tao = ps_po_pool.tile([128, 128], BF16, tag="p", name="tao")
                        nc.tensor.transpose(tao, ao, ident_bf)
                        nc.vector.tensor_copy(
                            out=aoT[:, 128 * j : 128 * (j + 1)], in_=tao
                        )
                    return aoT, q0

            def outproj_steps(aoT, q0, mm_pool=None, split=False):
                po_sb = po_pool.tile([128, DCH, QW], BF16, tag="po", name="po_sb")
                hd = DCH // 2

                def mk_mm(dc):
                    def step():
                        pool = mm_pool or ps_po_pool
                        ps_po = pool.tile(
                            [128, QW],
                            F32,
                            tag="o" if mm_pool is not None else "p",
                            name="ps_po",
                        )
                        nc.tensor.matmul(
                            ps_po,
                            lhsT=wo_sb[:, 128 * dc : 128 * (dc + 1)],
                            rhs=aoT,
                            start=True,
                            stop=True,
                        )
                        # steady-state drains stay on DVE (ACT is exp-bound);
                        # the tail (after the last attention chunk) splits
                        # DVE/ACT since ACT idles there
                        if split and dc % 2 == 1:
                            nc.scalar.copy(out=po_sb[:, dc, :], in_=ps_po)
                        else:
                            nc.vector.tensor_copy(out=po_sb[:, dc, :], in_=ps_po)

                    return step

                def mk_dma(half):
                    def step():
                        nc.sync.dma_start(
                            out=pot_d[:, :].rearrange("(a p) t -> p a t", p=128)[
                                :, half * hd : (half + 1) * hd, q0 : q0 + QW
                            ],
                            in_=po_sb[:, half * hd : (half + 1) * hd, :],
                        )

                    return step

                steps = []
                for half in range(2):
                    for dc in range(half * hd, (half + 1) * hd):
                        steps.append(mk_mm(dc))
                    steps.append(mk_dma(half))
                return steps

            # ---- Emission order: b0 proj, then b1 proj segments
            # interleaved with b0 attention chunks, then b1 attention.
            # Each chunk's out-projection is deferred past the next chunk's
            # kt loop so its PSUM->SBUF copies never block the PE stream.
            NTC = XC // 128

            xts = {}

            def get_xload(c):
                if c < NXC and c not in xts:
                    xts[c] = emit_xload(c)
                return xts.get(c)

            def emit_proj_chunk(c):
                xt_sb = get_xload(c)
                if c == 0:
                    emit_wqkv_half(1)
                get_xload(c + 1)
                ssq4 = lnpool.tile([128, 4 * NTC], F32, tag="ssq", name="ssq4")
                qkcps = [
                    emit_proj_tile(xt_sb, NTC * c + tt, ssq4) for tt in range(NTC)
                ]
                lnv = lnpool.tile([128, 4 * NTC], F32, tag="lnv", name="lnv")
                nc.scalar.activation(
                    out=lnv, in_=ssq4, func=AF.Ln, scale=1.0 / DH, bias=epsb[:, :]
                )
                nc.scalar.activation(
                    out=rstdall[:, NTC * c : NTC * (c + 1), :].rearrange(
                        "p a b -> p (a b)"
                    ),
                    in_=lnv,
                    func=AF.Exp,
                    scale=-0.5,
                    bias=ln8b[:, :],
                )
                for tt in range(NTC):
                    emit_proj_back(qkcps[tt], NTC * c + tt)

            # pipelined emission with a LAG: attention chunk i is emitted
            # after projection chunk i+LAG, so the exp stream starts early
            # but each chunk's LN-chain -> kT latency is hidden by the
            # in-flight projection work ahead of it
            LAG = 2
            wo_sb = const_pool.tile([CH, D], BF16, name="wo_sb")
            steps = []
            last_aoT = [None]
            NCH = B * QC

            def attn_of(i):
                return (i // QC, i % QC)

            for i in range(NCH + LAG):
                if i < NCH:
                    emit_proj_chunk(i)
                    if i == 0:
                        nc.sync.dma_start(out=wo_sb, in_=wot_d[:, :])
                ai = i - LAG
                if ai < 0:
                    continue
                b, qc = attn_of(ai)
                st = emit_attn_chunk(b, qc, steps)
                last_aoT[0] = st
                for s in steps:
                    s()
                if ai == NCH - 1:
                    steps = outproj_steps(*st, mm_pool=ps_o_pool, split=True)
                else:
                    steps = outproj_steps(*st)
            for s in steps:
                s()
            if dbg:
                nc.sync.dma_start(out=dbg_d["qT"][:, :], in_=qT)
                nc.sync.dma_start(out=dbg_d["kT"][:, :], in_=kT)
                nc.sync.dma_start(
                    out=dbg_d["vaug"][:, :],
                    in_=vaug.rearrange("p a b -> p (a b)"),
                )
                nc.sync.dma_start(
                    out=dbg_d["rstd"][:, :],
                    in_=rstdall.rearrange("p a b -> p (a b)"),
                )
                nc.sync.dma_start(out=dbg_d["aoT"][:, :], in_=last_aoT[0][0])

    _split_drain_waits(nc)
    return nc


_NC_CACHE = {}


def _get_nc(use_bias=False, dbg=False):
    key = (use_bias, dbg)
    if key not in _NC_CACHE:
        _NC_CACHE[key] = _build(use_bias, dbg=dbg)
    return _NC_CACHE[key]


def _bf16(a):
    return np.asarray(a, dtype=np.float32).astype(mybir.dt.np(BF16))


def _f8_pair(a):
    """Split fp32 array into (hi, lo) fp8e4m3 with hi+lo ~= a (0.05% resid)."""
    f8 = mybir.dt.np(mybir.dt.float8e4)
    hi = a.astype(f8)
    lo = (a - hi.astype(np.float32)).astype(f8)
    return hi, lo


def _prep_inputs(x, Wq, bq, Wk, bk, Wv, bv, Wo):
    xt = np.ascontiguousarray(x.reshape(T, D).T).astype(np.float32)
    xc = xt.reshape(DCH, 128, T).transpose(1, 0, 2)  # [128, DCH, T]
    xh, xl = _f8_pair(xc)
    xq = np.ascontiguousarray(
        np.stack([xh, xl], axis=2).reshape(128, DCH * 2 * T)
    )
    in_maps = []
    for c in range(NCORES):
        sl = slice(CH * c, CH * (c + 1))
        wq_c = np.array(Wq[sl, :], dtype=np.float32)
        bq_c = np.array(bq[sl], dtype=np.float32)
        wk_c = np.array(Wk[sl, :], dtype=np.float32)
        bk_c = np.array(bk[sl], dtype=np.float32)
        # fold the LayerNorm mean-subtraction (a linear map) into W and b
        for h in range(HPC):
            blk = slice(DH * h, DH * (h + 1))
            wq_c[blk, :] -= wq_c[blk, :].mean(axis=0, keepdims=True)
            bq_c[blk] -= bq_c[blk].mean()
            wk_c[blk, :] -= wk_c[blk, :].mean(axis=0, keepdims=True)
            bk_c[blk] -= bk_c[blk].mean()
        wv_c = np.array(Wv[sl, :], dtype=np.float32)
        bv_c = np.array(bv[sl], dtype=np.float32)
        wfull = np.ascontiguousarray(
            np.concatenate([wq_c, wk_c, wv_c], axis=0).T * WS
        )  # [D, 3CH], x32 into fp8-normal range
        wc = wfull.reshape(DCH, 128, 3 * CH).transpose(1, 0, 2)
        wh, wl = _f8_pair(wc)
        wqkvt = np.ascontiguousarray(
            np.stack([wh, wl], axis=2).reshape(128, DCH * 2 * 3 * CH)
        )
        bqkv = (np.concatenate([bq_c, bk_c, bv_c]) * WS)[None, :].astype(
            np.float32
        )
        wot = _bf16(np.ascontiguousarray(Wo[:, sl].T) / WS)
        in_maps.append({"xq": xq, "wqkvt": wqkvt, "bqkv": bqkv, "wot": wot})
    return in_maps


def kernel(x, mask, Wq, bq, Wk, bk, Wv, bv, Wo, bo, _trace=False):
    x = np.asarray(x, dtype=np.float32)
    in_maps = _prep_inputs(
        x,
        np.asarray(Wq),
        np.asarray(bq),
        np.asarray(Wk),
        np.asarray(bk),
        np.asarray(Wv),
        np.asarray(bv),
        np.asarray(Wo),
    )
    use_bias = bool(
        np.any(np.asarray(bq)) or np.any(np.asarray(bk)) or np.any(np.asarray(bv))
    )
    if not use_bias:
        for m in in_maps:
            del m["bqkv"]
    nc = _get_nc(use_bias)
    res = run_bass_kernel_spmd(
        nc, in_maps, core_ids=list(range(NCORES)), trace=_trace
    )
    pot = np.zeros((D, T), np.float64)
    for c in range(NCORES):
        pot += res.results[c]["pot"].astype(np.float64)
    out = pot.T.astype(np.float32) + np.asarray(bo, dtype=np.float32)[None, :]
    out = out.reshape(B, S, D)
    if _trace:
        return out, res
    return out

